# revision 33
# baseline (speedup 1.0000x reference)
"""Mamba block kernel for Trainium2 (8 NeuronCores).

Sharding: batch (2-way) x tensor-parallel over d_inner (4-way).
Core c handles batch c//4 and d_inner channels [(c%4)*512, (c%4+1)*512).
Host folds norm_w into in_proj, pre-adds hidden+residual (bf16), and sums
the 4 TP partial outputs per batch.

Device pipeline per core (one NEFF, phases overlap via Tile scheduling):
  A. RMSNorm of r=hid+res (ACT square-accumulate variance), PE-transpose
     via bf16 identity matmuls -> hT_all [1024, L] bf16 (k-major columns)
  B. in_proj x-half (k-outer bf16 matmuls, 1024-wide moving), causal
     depthwise conv as 4 shifted diag-matmuls on PE accumulating in PSUM
     (SiLU fused in the ACT eviction), x_proj partials
  D. AllReduce of bf16 x_dbl partials (groups [[0-3],[4-7]], DRAM bounce);
     the z-half of in_proj + SiLU runs under the collective's latency
  F. per d-chunk (software-pipelined):
       dt = softplus via ACT: u = Exp(dt_proj + bias), dt = Ln(u + 1)
       ub = dt*x -> 4 DRAM copies -> per-quad expansion DMAs (4 per quad)
       selective scan over 16 expanded tiles [128=(8 d x 16 n), L]:
         a = exp(dt*A): PE f32r replicate-matmul, ACT Exp w/ A scale
         b = ub_exp*B broadcast (DVE bf16 2x)
         h = tensor_tensor_scan on DVE (1x, the hard floor)
         hc = h*C (split DVE / GpSimd Pool to balance engine load)
         y = sel 0/1-matmuls accumulating 16 subtiles into one PSUM tile
       y2 = D*x + y (DVE stt), yg = y2*silu(z) (Pool)
  G. out_proj partial (yg-block stationary, wout moving) -> [L, 1024] f32
"""

import sys

sys.path.insert(0, "/opt/trn_rl_repo")

import numpy as np

import concourse.bacc as bacc
import concourse.tile as tile
from concourse import mybir
from concourse.bass_utils import run_bass_kernel_spmd

F32 = mybir.dt.float32
F32R = mybir.dt.float32r
BF16 = mybir.dt.bfloat16
AF = mybir.ActivationFunctionType
OP = mybir.AluOpType

D_MODEL = 1024
D_INNER = 2048
NST = 16          # d_state
DT_RANK = 64
DCONV = 4
BATCH = 2
L = 2048
EPS = 1e-5

N_CORES = 8
TPG = 4                    # tensor-parallel group size
DLOC = D_INNER // TPG      # 512 channels per core
DC = DLOC // 128           # 4 partition chunks of x-channels
KC = D_MODEL // 128        # 8 contraction chunks
RT = L // 128              # 16 row tiles
NSUB = 128 // NST          # 8 d-channels per expanded tile
SPC = 128 // NSUB          # 16 subtiles per d-chunk

# hc-mul engine assignment: subtile (d*SPC+s) goes to Pool unless in this set
HC_DVE = frozenset()


def _build():
    nc = bacc.Bacc("TRN2", target_bir_lowering=False, debug=False,
                   enable_asserts=True, num_devices=N_CORES)

    def din(name, shape, dt=F32):
        return nc.dram_tensor(name, shape, dt, kind="ExternalInput").ap()

    rin = din("rin", [L, D_MODEL], BF16)        # hid+res, host-added
    winx = din("winx", [D_MODEL, DLOC], BF16)   # in_proj_w[x-slice].T * nw
    winz = din("winz", [D_MODEL, DLOC], BF16)   # in_proj_w[z-slice].T * nw
    wxT = din("wxT", [DLOC, 96], BF16)          # x_proj_w[:, slice].T
    wdtT = din("wdtT", [DT_RANK, DLOC], BF16)   # dt_proj_w[slice].T
    woutT = din("woutT", [DLOC, D_MODEL], BF16)  # out_proj_w[:, slice].T
    convd = din("convd", [128, DC * DCONV * 128], BF16)  # diag stationaries
    convb = din("convb", [128, DC])
    dtb = din("dtb", [128, DC])
    dpar = din("dpar", [128, DC])
    a_sc = din("a_sc", [128, DC * SPC])         # per-tile A scale column
    selm = din("selm", [128, SPC * 128], BF16)  # 16 selection matrices
    expm = din("expm", [128, SPC * 128], F32R)  # 16 expansion matrices
    identb = din("identb", [128, 128], BF16)

    out_part = nc.dram_tensor("out_part", [L, D_MODEL], F32,
                              kind="ExternalOutput").ap()

    with tile.TileContext(nc) as tc:
        cst = tc.alloc_tile_pool(name="cst", bufs=1)
        dram = tc.alloc_tile_pool(name="dram", bufs=1, space="DRAM")
        pW = tc.alloc_tile_pool(name="pW", bufs=1)

        # ---- constants / weights to SBUF ----
        convd_sb = pW.tile([128, DC * DCONV * 128], BF16)
        nc.sync.dma_start(convd_sb[:], convd[:])
        convb_sb = cst.tile([128, DC], F32)
        nc.sync.dma_start(convb_sb[:], convb[:])
        dtb_sb = cst.tile([128, DC], F32)
        nc.sync.dma_start(dtb_sb[:], dtb[:])
        dpar_sb = cst.tile([128, DC], F32)
        nc.sync.dma_start(dpar_sb[:], dpar[:])
        asc_sb = cst.tile([128, DC * SPC], F32)
        nc.sync.dma_start(asc_sb[:], a_sc[:])
        sel_sb = cst.tile([128, SPC * 128], BF16)
        nc.sync.dma_start(sel_sb[:], selm[:])
        exp_sb = cst.tile([128, SPC * 128], F32R)
        nc.sync.dma_start(exp_sb[:], expm[:])
        id_sb = cst.tile([128, 128], BF16)
        nc.sync.dma_start(id_sb[:], identb[:])
        eps_sb = cst.tile([128, 1], F32)
        nc.vector.memset(eps_sb[:], EPS)
        wx_sb = [cst.tile([128, 96], BF16, tag=f"wx{d}", name=f"wx{d}")
                 for d in range(DC)]
        for d in range(DC):
            nc.sync.dma_start(wx_sb[d][:], wxT[128 * d:128 * (d + 1), :])
        wdt_sb = cst.tile([DT_RANK, DLOC], BF16)
        nc.sync.dma_start(wdt_sb[:], wdtT[:])
        wout_sb = [cst.tile([128, D_MODEL], BF16, tag=f"wo{d}", name=f"wo{d}")
                   for d in range(DC)]
        for d in range(DC):
            nc.sync.dma_start(wout_sb[d][:], woutT[128 * d:128 * (d + 1), :])
        winx_sb = [pW.tile([128, DLOC], BF16, tag=f"winx{k}", name=f"winx{k}")
                   for k in range(KC)]
        winz_sb = [pW.tile([128, DLOC], BF16, tag=f"winz{k}", name=f"winz{k}")
                   for k in range(KC)]
        hT_all = pW.tile([128, KC * L], BF16)
        hT_v = hT_all[:].rearrange("p (k t) -> p k t", k=KC)

        # ====== Phases A+B interleaved: RMSNorm/transpose windows feed
        # in_proj x windows; conv on PE; x_proj -> AllReduce; z under AR ====
        for k in range(KC):
            nc.sync.dma_start(winx_sb[k][:], winx[128 * k:128 * (k + 1), :])
            nc.sync.dma_start(winz_sb[k][:], winz[128 * k:128 * (k + 1), :])

        pBC = tc.alloc_tile_pool(name="pBC", bufs=1, side="right")
        zg = [pBC.tile([128, L], BF16, tag=f"zg{d}", name=f"zg{d}")
              for d in range(DC)]
        xb = [pBC.tile([128, L], BF16, tag=f"xb{d}", name=f"xb{d}")
              for d in range(DC)]
        pDE = tc.alloc_tile_pool(name="pDE", bufs=1, side="right")
        xdbl_p = pDE.tile([96, L], BF16)
        xdbl_sb = pDE.tile([96, L], BF16)
        bbc = pDE.tile([128, L], BF16)
        cbc = pDE.tile([128, L], BF16)
        pX = tc.alloc_tile_pool(name="pX", bufs=1, side="right")
        xpad = [pX.tile([128, L + DCONV], BF16, tag=f"xpad{d}",
                        name=f"xpad{d}") for d in range(DC)]
        for d in range(DC):
            nc.vector.memset(xpad[d][:, 0:DCONV - 1], 0.0)

        ps_fr = tc.alloc_tile_pool(name="ps_fr", bufs=1, space="PSUM")
        WN = L // 512   # 4 in_proj windows of 512 timesteps

        with tc.tile_pool(name="pA", bufs=4) as pA, \
             tc.tile_pool(name="pA2", bufs=3) as pA2:

            def emit_rt(rt):
                t0 = 128 * rt
                ld = pA.tile([128, D_MODEL], BF16, tag="ld")
                nc.scalar.dma_start(ld[:], rin[t0:t0 + 128, :])
                sq = pA2.tile([128, D_MODEL], BF16, tag="sq", bufs=1)
                st = pA2.tile([128, 1], F32, tag="st")
                nc.scalar.activation(sq[:], ld[:], AF.Square, accum_out=st[:])
                sg = pA2.tile([128, 1], F32, tag="sg")
                nc.scalar.activation(sg[:], st[:], AF.Sqrt,
                                     bias=eps_sb[:], scale=1.0 / D_MODEL)
                rstd = pA2.tile([128, 1], F32, tag="rstd")
                nc.vector.reciprocal(rstd[:], sg[:])
                hrow = pA2.tile([128, D_MODEL], BF16, tag="hrow")
                nc.vector.tensor_scalar_mul(hrow[:], ld[:], rstd[:])
                for c in range(2):
                    pt = ps_fr.tile([128, 512], BF16, tag="tr", bufs=2,
                                    name="pt")
                    for j in range(4):
                        k = 4 * c + j
                        nc.tensor.transpose(pt[:, 128 * j:128 * (j + 1)],
                                            hrow[:, 128 * k:128 * (k + 1)],
                                            id_sb[:])
                    dst = hT_v[:, 4 * c:4 * (c + 1), t0:t0 + 128]
                    psrc = pt[:].rearrange("p (k t) -> p k t", k=4)
                    nc.vector.tensor_copy(dst, psrc)

            def emit_proj_window(w_sb, w, dest_fn):
                for d in range(DC):
                    pm = ps_fr.tile([128, 512], F32, tag=f"px{d}",
                                    name="pm")
                    for k in range(KC):
                        nc.tensor.matmul(
                            pm[:], w_sb[k][:, 128 * d:128 * (d + 1)],
                            hT_v[:, k, 512 * w:512 * (w + 1)],
                            start=(k == 0), stop=(k == KC - 1))
                    dest_fn(d, w, pm)

            def evict_x(d, w, pm):
                o = DCONV - 1 + 512 * w
                nc.vector.tensor_copy(xpad[d][:, o:o + 512], pm[:])

            def emit_conv_w(d, w):
                # causal conv window: xpad cols [512w, 512w+512+3) suffice
                pm = ps_fr.tile([128, 512], F32, tag="pxp", bufs=2,
                                name="pm")
                for k in range(DCONV):
                    o = 512 * w + k
                    nc.tensor.matmul(
                        pm[:],
                        convd_sb[:, 128 * (DCONV * d + k):
                                 128 * (DCONV * d + k + 1)],
                        xpad[d][:, o:o + 512],
                        start=(k == 0), stop=(k == DCONV - 1))
                nc.scalar.activation(xb[d][:, 512 * w:512 * (w + 1)],
                                     pm[:], AF.Silu,
                                     bias=convb_sb[:, d:d + 1])

            def emit_xproj_w(w):
                pm = ps_fr.tile([128, 512], F32, tag="pxp", bufs=2,
                                name="pm")
                for d in range(DC):
                    nc.tensor.matmul(pm[0:96, :], wx_sb[d][:],
                                     xb[d][:, 512 * w:512 * (w + 1)],
                                     start=(d == 0), stop=(d == DC - 1))
                nc.vector.tensor_copy(xdbl_p[:, 512 * w:512 * (w + 1)],
                                      pm[0:96, :])

            for w in range(WN):
                for rt in range(4 * w, 4 * w + 4):
                    emit_rt(rt)
                emit_proj_window(winx_sb, w, evict_x)
                for d in range(DC):
                    emit_conv_w(d, w)
                emit_xproj_w(w)
        pX.release()

        # ====== Phase D: AllReduce (bf16); z-half fills the latency ======
        # bounce path on the DVE DMA queue so it isn't stuck behind SP's
        # paced rin loads
        bounce_i = dram.tile([96, L], BF16)
        bounce_o = dram.tile([96, L], BF16)
        nc.gpsimd.dma_start(bounce_i[:], xdbl_p[:])
        nc.gpsimd.collective_compute(
            "AllReduce", OP.add,
            replica_groups=[[0, 1, 2, 3], [4, 5, 6, 7]],
            ins=[bounce_i.opt()], outs=[bounce_o.opt()])
        nc.gpsimd.dma_start(xdbl_sb[:], bounce_o[:])

        def evict_z(d, w, pm):
            nc.scalar.activation(zg[d][:, 512 * w:512 * (w + 1)], pm[:],
                                 AF.Silu)

        with tc.tile_pool(name="pZ", bufs=1) as _pz:
            for w in range(WN):
                for d in range(DC):
                    pm = ps_fr.tile([128, 512], F32, tag=f"px{d}", name="pm")
                    for k in range(KC):
                        nc.tensor.matmul(
                            pm[:], winz_sb[k][:, 128 * d:128 * (d + 1)],
                            hT_v[:, k, 512 * w:512 * (w + 1)],
                            start=(k == 0), stop=(k == KC - 1))
                    evict_z(d, w, pm)

        dtlow = xdbl_sb[0:64, :]
        for i in range(NSUB):
            nc.gpsimd.dma_start(bbc[NST * i:NST * (i + 1), :],
                                xdbl_sb[64:80, :])
            nc.gpsimd.dma_start(cbc[NST * i:NST * (i + 1), :],
                                xdbl_sb[80:96, :])
        pW.release()
        ps_fr.release()

        # ====== Phase F: dt path + selective scan (fused per d) ======
        pY = tc.alloc_tile_pool(name="pY", bufs=1, side="right")
        yg = [pY.tile([128, L], BF16, tag=f"yg{d}", name=f"yg{d}")
              for d in range(DC)]
        with tc.tile_pool(name="pF", bufs=2) as pF, \
             tc.tile_pool(name="pQ", bufs=2) as pQ, \
             tc.tile_pool(name="ps_y", bufs=1, space="PSUM") as ps_y, \
             tc.tile_pool(name="ps_f", bufs=2, space="PSUM") as ps_f:
            dt_ds = {}
            ub_ds = {}
            prep_u = {}

            def emit_prep_mm(d):
                # dt_raw matmuls + u = Exp(.+bias)
                u_t = pF.tile([128, L], F32, tag="u_t", bufs=1, name="u_t")
                for t2 in range(2):
                    pm = ps_f.tile([128, 1024], F32, tag="pa", name="pa")
                    for h in range(2):
                        o = 1024 * t2 + 512 * h
                        nc.tensor.matmul(
                            pm[:, 512 * h:512 * (h + 1)],
                            wdt_sb[:, 128 * d:128 * (d + 1)],
                            dtlow[:, o:o + 512], start=True, stop=True,
                            skip_group_check=True)
                    nc.scalar.activation(u_t[:, 1024 * t2:1024 * (t2 + 1)],
                                         pm[:], AF.Exp,
                                         bias=dtb_sb[:, d:d + 1])
                return u_t

            def emit_prep_ln(d, u_t):
                dt_d = pF.tile([128, L], F32R, tag="dt_d", bufs=2,
                               name="dt_d")
                nc.scalar.activation(dt_d[:], u_t[:], AF.Ln, bias=1.0)
                dt_ds[d] = dt_d

            def emit_prep_ub(d):
                ub_d = pF.tile([128, L], BF16, tag="ub_d", bufs=1,
                               name="ub_d")
                nc.vector.tensor_mul(ub_d[:], dt_ds[d][:].bitcast(F32),
                                     xb[d][:])
                ub_sc = dram.tile([128, L], BF16, tag="ub_sc", bufs=2,
                                  name="ub_sc")
                nc.sync.dma_start(ub_sc[:], ub_d[:])
                ub_ds[d] = ub_sc

            def emit_prep(d):
                emit_prep_ln(d, emit_prep_mm(d))
                emit_prep_ub(d)

            emit_prep(0)

            NG = DC * SPC  # 64 global subtiles
            quads = {}
            a_ts = {}
            h_ts = {}
            hc_ts = {}
            ypsums = {}

            def emit_quad_reads(qg):
                # quad qg covers subtiles 4qg..4qg+3 of d = qg//4
                quad = pQ.tile([128, 4 * L], BF16, tag="quad", bufs=3)
                quad_v = quad[:].rearrange(
                    "(i n) (s t) -> i n s t", i=NSUB, s=4)
                src_r = ub_ds[qg // 4][32 * (qg % 4):32 * (qg % 4 + 1),
                                       :].rearrange("(s i) t -> i s t", s=4)
                for n in range(NST):
                    nc.sync.dma_start(quad_v[:, n, :, :], src_r)
                quads[qg] = quad

            def emit_S0(g):
                # a = exp(dt*A): PE replicate-matmul + ACT Exp
                d, sidx = divmod(g, SPC)
                dt_d = dt_ds[d]
                a_t = pF.tile([128, L], F32, tag="a", bufs=2)
                for t2 in range(2):
                    pm = ps_f.tile([128, 1024], F32, tag="pa")
                    for h in range(2):
                        o = 1024 * t2 + 512 * h
                        nc.tensor.matmul(
                            pm[:, 512 * h:512 * (h + 1)],
                            exp_sb[:, 128 * sidx:128 * (sidx + 1)],
                            dt_d[:, o:o + 512], start=True, stop=True,
                            skip_group_check=True)
                    nc.scalar.activation(
                        a_t[:, 1024 * t2:1024 * (t2 + 1)], pm[:], AF.Exp,
                        scale=asc_sb[:, d * SPC + sidx:d * SPC + sidx + 1])
                a_ts[g] = a_t

            def emit_S12(g):
                # b = ub_exp*B (DVE), h = scan (DVE), hc = h*C (Pool/DVE)
                sq = g % 4
                quad = quads[g // 4]
                b_t = pF.tile([128, L], BF16, tag="b")
                nc.vector.tensor_mul(b_t[:], quad[:, L * sq:L * (sq + 1)],
                                     bbc[:])
                h_t = pF.tile([128, L], BF16, tag="h")
                nc.vector.tensor_tensor_scan(h_t[:], a_ts.pop(g)[:], b_t[:],
                                             0.0, OP.mult, OP.add)
                hc = pF.tile([128, L], BF16, tag="hc")
                if g % 9 == 4:
                    nc.vector.tensor_mul(hc[:], h_t[:], cbc[:])
                else:
                    nc.gpsimd.tensor_mul(hc[:], h_t[:], cbc[:])
                hc_ts[g] = hc

            def emit_S4(g):
                d, sidx = divmod(g, SPC)
                if sidx == 0:
                    ypsums[d] = ps_y.tile([128, L], F32, tag="ypsum",
                                          name="ypsum")
                hc = hc_ts.pop(g)
                for tq in range(4):
                    nc.tensor.matmul(
                        ypsums[d][:, 512 * tq:512 * (tq + 1)],
                        sel_sb[:, 128 * sidx:128 * (sidx + 1)],
                        hc[:, 512 * tq:512 * (tq + 1)],
                        start=(sidx == 0), stop=(sidx == SPC - 1),
                        skip_group_check=True)
                if sidx == SPC - 1:
                    # y2 = D*x + y ;  yg = y2 * silu(z)
                    for hh in range(2):
                        o = 1024 * hh
                        y2 = pF.tile([128, 1024], F32, tag="y2", bufs=1)
                        nc.vector.scalar_tensor_tensor(
                            y2[:], xb[d][:, o:o + 1024], dpar_sb[:, d:d + 1],
                            ypsums[d][:, o:o + 1024], OP.mult, OP.add)
                        nc.vector.tensor_mul(yg[d][:, o:o + 1024], y2[:],
                                               zg[d][:, o:o + 1024])

            emit_quad_reads(0)
            emit_quad_reads(1)
            for step in range(NG + 2):
                g0, g1, g2 = step, step - 1, step - 2
                if g0 < NG:
                    if g0 % SPC == 8 and g0 // SPC + 1 < DC:
                        emit_prep(g0 // SPC + 1)
                    gq = g0 + 6
                    if gq % 4 == 0 and gq < NG:
                        emit_quad_reads(gq // 4)
                    emit_S0(g0)
                if 0 <= g1 < NG:
                    emit_S12(g1)
                if 0 <= g2 < NG:
                    emit_S4(g2)
        # ====== Phase G: out_proj ======
        with tc.tile_pool(name="pG", bufs=3) as pG, \
             tc.tile_pool(name="ps_g", bufs=2, space="PSUM") as ps_g:
            for tb in range(RT):
                pm = ps_g.tile([128, D_MODEL], F32, tag="pmG")
                for d in range(DC):
                    for h in range(2):
                        nc.tensor.matmul(
                            pm[:, 512 * h:512 * (h + 1)],
                            yg[d][:, 128 * tb:128 * (tb + 1)],
                            wout_sb[d][:, 512 * h:512 * (h + 1)],
                            start=(d == 0), stop=(d == DC - 1))
                osb = pG.tile([128, D_MODEL], F32, tag="osb")
                nc.scalar.activation(osb[:], pm[:], AF.Copy)
                nc.sync.dma_start(out_part[128 * tb:128 * (tb + 1), :],
                                  osb[:])
        pY.release()
        pDE.release()
        pBC.release()
        cst.release()
        dram.release()
    nc.compile()

    return nc


_NC_CACHE = None


def _get_nc():
    global _NC_CACHE
    if _NC_CACHE is None:
        _NC_CACHE = _build()
    return _NC_CACHE


def kernel(input_ids=None, hidden_states=None, residual=None, norm_w=None,
           in_proj_w=None, conv_w=None, conv_b=None, x_proj_w=None,
           dt_proj_w=None, dt_proj_b=None, A_log=None, D_param=None,
           out_proj_w=None, **kwargs):
    import ml_dtypes
    bf16 = np.dtype(ml_dtypes.bfloat16)

    hs = np.asarray(hidden_states, np.float32)
    rs = np.asarray(residual, np.float32)
    ipw = np.asarray(in_proj_w, np.float32)
    cw = np.asarray(conv_w, np.float32)
    cb = np.asarray(conv_b, np.float32)
    xpw = np.asarray(x_proj_w, np.float32)
    dpw = np.asarray(dt_proj_w, np.float32)
    dpb = np.asarray(dt_proj_b, np.float32)
    al = np.asarray(A_log, np.float32)
    dpr = np.asarray(D_param, np.float32)
    opw = np.asarray(out_proj_w, np.float32)
    nw = np.asarray(norm_w, np.float32)

    r_full = hs + rs                               # host-side residual add

    def colpack(v):  # [DLOC] -> [128, DC], col d = v[d*128:(d+1)*128]
        return np.ascontiguousarray(v.reshape(DC, 128).T).astype(np.float32)

    selm = np.zeros((128, SPC * 128), np.float32)
    expm = np.zeros((128, SPC * 128), np.float32)
    for s in range(SPC):
        for i in range(NSUB):
            m = s * NSUB + i
            for n in range(NST):
                p = i * NST + n
                selm[p, s * 128 + m] = 1.0
                expm[m, s * 128 + p] = 1.0
    identb = np.eye(128, dtype=np.float32)

    nc = _get_nc()
    in_maps = []
    for c in range(N_CORES):
        b, k = c // TPG, c % TPG
        sl = slice(k * DLOC, (k + 1) * DLOC)
        slz = slice(D_INNER + k * DLOC, D_INNER + (k + 1) * DLOC)

        conv4 = cw[sl, 0, :]                       # [DLOC, 4]
        convd = np.zeros((128, DC * DCONV * 128), np.float32)
        for d in range(DC):
            for kk in range(DCONV):
                blk = DCONV * d + kk
                np.fill_diagonal(
                    convd[:, 128 * blk:128 * (blk + 1)],
                    conv4[128 * d:128 * (d + 1), kk])

        A = -np.exp(al[sl])                        # [DLOC, 16]
        a_sc = np.zeros((128, DC * SPC), np.float32)
        for d in range(DC):
            for s in range(SPC):
                rows = A[d * 128 + s * NSUB: d * 128 + (s + 1) * NSUB, :]
                a_sc[:, d * SPC + s] = rows.reshape(128)

        in_maps.append(dict(
            rin=r_full[b].astype(bf16),
            winx=np.ascontiguousarray(ipw[sl].T * nw[:, None]).astype(bf16),
            winz=np.ascontiguousarray(ipw[slz].T * nw[:, None]).astype(bf16),
            wxT=np.ascontiguousarray(xpw[:, sl].T).astype(bf16),
            wdtT=np.ascontiguousarray(dpw[sl].T).astype(bf16),
            woutT=np.ascontiguousarray(opw[:, sl].T).astype(bf16),
            convd=convd.astype(bf16),
            convb=colpack(cb[sl]),
            dtb=colpack(dpb[sl]),
            dpar=colpack(dpr[sl]),
            a_sc=a_sc,
            selm=selm.astype(bf16),
            expm=expm,
            identb=identb.astype(bf16),
        ))

    res = run_bass_kernel_spmd(nc, in_maps, core_ids=list(range(N_CORES)))
    outs = [res.results[c]["out_part"] for c in range(N_CORES)]
    full = np.stack([
        sum(outs[b * TPG + k] for k in range(TPG)) for b in range(BATCH)
    ]).astype(np.float32)
    return full


# revision 34
# speedup vs baseline: 1.0007x; 1.0007x over previous
"""Mamba block kernel for Trainium2 (8 NeuronCores).

Sharding: batch (2-way) x tensor-parallel over d_inner (4-way).
Core c handles batch c//4 and d_inner channels [(c%4)*512, (c%4+1)*512).
Host folds norm_w into in_proj, pre-adds hidden+residual (bf16), and sums
the 4 TP partial outputs per batch.

Device pipeline per core (one NEFF, phases overlap via Tile scheduling):
  A. RMSNorm of r=hid+res (ACT square-accumulate variance), PE-transpose
     via bf16 identity matmuls -> hT_all [1024, L] bf16 (k-major columns)
  B. in_proj x-half (k-outer bf16 matmuls, 1024-wide moving), causal
     depthwise conv as 4 shifted diag-matmuls on PE accumulating in PSUM
     (SiLU fused in the ACT eviction), x_proj partials
  D. AllReduce of bf16 x_dbl partials (groups [[0-3],[4-7]], DRAM bounce);
     the z-half of in_proj + SiLU runs under the collective's latency
  F. per d-chunk (software-pipelined):
       dt = softplus via ACT: u = Exp(dt_proj + bias), dt = Ln(u + 1)
       ub = dt*x -> 4 DRAM copies -> per-quad expansion DMAs (4 per quad)
       selective scan over 16 expanded tiles [128=(8 d x 16 n), L]:
         a = exp(dt*A): PE f32r replicate-matmul, ACT Exp w/ A scale
         b = ub_exp*B broadcast (DVE bf16 2x)
         h = tensor_tensor_scan on DVE (1x, the hard floor)
         hc = h*C (split DVE / GpSimd Pool to balance engine load)
         y = sel 0/1-matmuls accumulating 16 subtiles into one PSUM tile
       y2 = D*x + y (DVE stt), yg = y2*silu(z) (Pool)
  G. out_proj partial (yg-block stationary, wout moving) -> [L, 1024] f32
"""

import sys

sys.path.insert(0, "/opt/trn_rl_repo")

import numpy as np

import concourse.bacc as bacc
import concourse.tile as tile
from concourse import mybir
from concourse.bass_utils import run_bass_kernel_spmd

F32 = mybir.dt.float32
F32R = mybir.dt.float32r
BF16 = mybir.dt.bfloat16
AF = mybir.ActivationFunctionType
OP = mybir.AluOpType

D_MODEL = 1024
D_INNER = 2048
NST = 16          # d_state
DT_RANK = 64
DCONV = 4
BATCH = 2
L = 2048
EPS = 1e-5

N_CORES = 8
TPG = 4                    # tensor-parallel group size
DLOC = D_INNER // TPG      # 512 channels per core
DC = DLOC // 128           # 4 partition chunks of x-channels
KC = D_MODEL // 128        # 8 contraction chunks
RT = L // 128              # 16 row tiles
NSUB = 128 // NST          # 8 d-channels per expanded tile
SPC = 128 // NSUB          # 16 subtiles per d-chunk

# hc-mul engine assignment: subtile (d*SPC+s) goes to Pool unless in this set
HC_DVE = frozenset()


def _build():
    nc = bacc.Bacc("TRN2", target_bir_lowering=False, debug=False,
                   enable_asserts=True, num_devices=N_CORES)

    def din(name, shape, dt=F32):
        return nc.dram_tensor(name, shape, dt, kind="ExternalInput").ap()

    rin = din("rin", [L, D_MODEL], BF16)        # hid+res, host-added
    winx = din("winx", [D_MODEL, DLOC], BF16)   # in_proj_w[x-slice].T * nw
    winz = din("winz", [D_MODEL, DLOC], BF16)   # in_proj_w[z-slice].T * nw
    wxT = din("wxT", [DLOC, 96], BF16)          # x_proj_w[:, slice].T
    wdtT = din("wdtT", [DT_RANK, DLOC], BF16)   # dt_proj_w[slice].T
    woutT = din("woutT", [DLOC, D_MODEL], BF16)  # out_proj_w[:, slice].T
    convd = din("convd", [128, DC * DCONV * 128], BF16)  # diag stationaries
    convb = din("convb", [128, DC])
    dtb = din("dtb", [128, DC])
    dpar = din("dpar", [128, DC])
    a_sc = din("a_sc", [128, DC * SPC])         # per-tile A scale column
    selm = din("selm", [128, SPC * 128], BF16)  # 16 selection matrices
    expm = din("expm", [128, SPC * 128], F32R)  # 16 expansion matrices
    identb = din("identb", [128, 128], BF16)

    out_part = nc.dram_tensor("out_part", [L, D_MODEL], F32,
                              kind="ExternalOutput").ap()

    with tile.TileContext(nc) as tc:
        cst = tc.alloc_tile_pool(name="cst", bufs=1)
        dram = tc.alloc_tile_pool(name="dram", bufs=1, space="DRAM")
        pW = tc.alloc_tile_pool(name="pW", bufs=1)

        # ---- constants / weights to SBUF ----
        convd_sb = cst.tile([128, DC * DCONV * 128], BF16)
        nc.sync.dma_start(convd_sb[:], convd[:])
        convb_sb = cst.tile([128, DC], F32)
        nc.sync.dma_start(convb_sb[:], convb[:])
        dtb_sb = cst.tile([128, DC], F32)
        nc.sync.dma_start(dtb_sb[:], dtb[:])
        dpar_sb = cst.tile([128, DC], F32)
        nc.sync.dma_start(dpar_sb[:], dpar[:])
        asc_sb = cst.tile([128, DC * SPC], F32)
        nc.sync.dma_start(asc_sb[:], a_sc[:])
        sel_sb = cst.tile([128, SPC * 128], BF16)
        nc.sync.dma_start(sel_sb[:], selm[:])
        exp_sb = cst.tile([128, SPC * 128], F32R)
        nc.sync.dma_start(exp_sb[:], expm[:])
        id_sb = cst.tile([128, 128], BF16)
        nc.sync.dma_start(id_sb[:], identb[:])
        eps_sb = cst.tile([128, 1], F32)
        nc.vector.memset(eps_sb[:], EPS)
        wx_sb = [cst.tile([128, 96], BF16, tag=f"wx{d}", name=f"wx{d}")
                 for d in range(DC)]
        for d in range(DC):
            nc.sync.dma_start(wx_sb[d][:], wxT[128 * d:128 * (d + 1), :])
        wdt_sb = cst.tile([DT_RANK, DLOC], BF16)
        nc.sync.dma_start(wdt_sb[:], wdtT[:])
        wout_sb = [cst.tile([128, D_MODEL], BF16, tag=f"wo{d}", name=f"wo{d}")
                   for d in range(DC)]
        for d in range(DC):
            nc.sync.dma_start(wout_sb[d][:], woutT[128 * d:128 * (d + 1), :])
        winx_sb = [pW.tile([128, DLOC], BF16, tag=f"winx{k}", name=f"winx{k}")
                   for k in range(KC)]
        winz_sb = [pW.tile([128, DLOC], BF16, tag=f"winz{k}", name=f"winz{k}")
                   for k in range(KC)]
        hT_all = pW.tile([128, KC * L], BF16)
        hT_v = hT_all[:].rearrange("p (k t) -> p k t", k=KC)

        # ====== Phases A+B interleaved: RMSNorm/transpose windows feed
        # in_proj x windows; conv on PE; x_proj -> AllReduce; z under AR ====
        for k in range(KC):
            nc.sync.dma_start(winx_sb[k][:], winx[128 * k:128 * (k + 1), :])
            nc.sync.dma_start(winz_sb[k][:], winz[128 * k:128 * (k + 1), :])

        pBC = tc.alloc_tile_pool(name="pBC", bufs=1, side="right")
        zg = [pBC.tile([128, L], BF16, tag=f"zg{d}", name=f"zg{d}")
              for d in range(DC)]
        xb = [pBC.tile([128, L], BF16, tag=f"xb{d}", name=f"xb{d}")
              for d in range(DC)]
        pDE = tc.alloc_tile_pool(name="pDE", bufs=1, side="right")
        xdbl_p = pDE.tile([96, L], BF16)
        xdbl_sb = pDE.tile([96, L], BF16)
        bbc = pDE.tile([128, L], BF16)
        cbc = pDE.tile([128, L], BF16)
        pX = tc.alloc_tile_pool(name="pX", bufs=1, side="right")
        xpad = [pX.tile([128, L + DCONV], BF16, tag=f"xpad{d}",
                        name=f"xpad{d}") for d in range(DC)]
        for d in range(DC):
            nc.vector.memset(xpad[d][:, 0:DCONV - 1], 0.0)

        ps_fr = tc.alloc_tile_pool(name="ps_fr", bufs=1, space="PSUM")
        WN = L // 512   # 4 in_proj windows of 512 timesteps

        with tc.tile_pool(name="pA", bufs=4) as pA, \
             tc.tile_pool(name="pA2", bufs=3) as pA2:

            def emit_rt(rt):
                t0 = 128 * rt
                ld = pA.tile([128, D_MODEL], BF16, tag="ld")
                nc.scalar.dma_start(ld[:], rin[t0:t0 + 128, :])
                sq = pA2.tile([128, D_MODEL], BF16, tag="sq", bufs=1)
                st = pA2.tile([128, 1], F32, tag="st")
                nc.scalar.activation(sq[:], ld[:], AF.Square, accum_out=st[:])
                sg = pA2.tile([128, 1], F32, tag="sg")
                nc.scalar.activation(sg[:], st[:], AF.Sqrt,
                                     bias=eps_sb[:], scale=1.0 / D_MODEL)
                rstd = pA2.tile([128, 1], F32, tag="rstd")
                nc.vector.reciprocal(rstd[:], sg[:])
                hrow = pA2.tile([128, D_MODEL], BF16, tag="hrow")
                nc.vector.tensor_scalar_mul(hrow[:], ld[:], rstd[:])
                for c in range(2):
                    pt = ps_fr.tile([128, 512], BF16, tag="tr", bufs=2,
                                    name="pt")
                    for j in range(4):
                        k = 4 * c + j
                        nc.tensor.transpose(pt[:, 128 * j:128 * (j + 1)],
                                            hrow[:, 128 * k:128 * (k + 1)],
                                            id_sb[:])
                    dst = hT_v[:, 4 * c:4 * (c + 1), t0:t0 + 128]
                    psrc = pt[:].rearrange("p (k t) -> p k t", k=4)
                    nc.vector.tensor_copy(dst, psrc)

            def emit_proj_window(w_sb, w, dest_fn):
                for d in range(DC):
                    pm = ps_fr.tile([128, 512], F32, tag=f"px{d}",
                                    name="pm")
                    for k in range(KC):
                        nc.tensor.matmul(
                            pm[:], w_sb[k][:, 128 * d:128 * (d + 1)],
                            hT_v[:, k, 512 * w:512 * (w + 1)],
                            start=(k == 0), stop=(k == KC - 1))
                    dest_fn(d, w, pm)

            def evict_x(d, w, pm):
                o = DCONV - 1 + 512 * w
                nc.vector.tensor_copy(xpad[d][:, o:o + 512], pm[:])

            def emit_conv_w(d, w):
                # causal conv window: xpad cols [512w, 512w+512+3) suffice
                pm = ps_fr.tile([128, 512], F32, tag="pxp", bufs=2,
                                name="pm")
                for k in range(DCONV):
                    o = 512 * w + k
                    nc.tensor.matmul(
                        pm[:],
                        convd_sb[:, 128 * (DCONV * d + k):
                                 128 * (DCONV * d + k + 1)],
                        xpad[d][:, o:o + 512],
                        start=(k == 0), stop=(k == DCONV - 1))
                nc.scalar.activation(xb[d][:, 512 * w:512 * (w + 1)],
                                     pm[:], AF.Silu,
                                     bias=convb_sb[:, d:d + 1])

            def emit_xproj_w(w):
                pm = ps_fr.tile([128, 512], F32, tag="pxp", bufs=2,
                                name="pm")
                for d in range(DC):
                    nc.tensor.matmul(pm[0:96, :], wx_sb[d][:],
                                     xb[d][:, 512 * w:512 * (w + 1)],
                                     start=(d == 0), stop=(d == DC - 1))
                nc.vector.tensor_copy(xdbl_p[:, 512 * w:512 * (w + 1)],
                                      pm[0:96, :])

            for w in range(WN):
                for rt in range(4 * w, 4 * w + 4):
                    emit_rt(rt)
                emit_proj_window(winx_sb, w, evict_x)
                for d in range(DC):
                    emit_conv_w(d, w)
                emit_xproj_w(w)
        pX.release()

        # ====== Phase D: AllReduce (bf16); z-half fills the latency ======
        # bounce path on the DVE DMA queue so it isn't stuck behind SP's
        # paced rin loads
        bounce_i = dram.tile([96, L], BF16)
        bounce_o = dram.tile([96, L], BF16)
        nc.gpsimd.dma_start(bounce_i[:], xdbl_p[:])
        nc.gpsimd.collective_compute(
            "AllReduce", OP.add,
            replica_groups=[[0, 1, 2, 3], [4, 5, 6, 7]],
            ins=[bounce_i.opt()], outs=[bounce_o.opt()])
        nc.gpsimd.dma_start(xdbl_sb[:], bounce_o[:])

        def evict_z(d, w, pm):
            nc.scalar.activation(zg[d][:, 512 * w:512 * (w + 1)], pm[:],
                                 AF.Silu)

        with tc.tile_pool(name="pZ", bufs=1) as _pz:
            for w in range(WN):
                for d in range(DC):
                    pm = ps_fr.tile([128, 512], F32, tag=f"px{d}", name="pm")
                    for k in range(KC):
                        nc.tensor.matmul(
                            pm[:], winz_sb[k][:, 128 * d:128 * (d + 1)],
                            hT_v[:, k, 512 * w:512 * (w + 1)],
                            start=(k == 0), stop=(k == KC - 1))
                    evict_z(d, w, pm)

        dtlow = xdbl_sb[0:64, :]
        for i in range(NSUB):
            nc.gpsimd.dma_start(bbc[NST * i:NST * (i + 1), :],
                                xdbl_sb[64:80, :])
            nc.gpsimd.dma_start(cbc[NST * i:NST * (i + 1), :],
                                xdbl_sb[80:96, :])
        pW.release()
        ps_fr.release()

        # ====== Phase F: dt path + selective scan (fused per d) ======
        pY = tc.alloc_tile_pool(name="pY", bufs=1, side="right")
        yg = [pY.tile([128, L], BF16, tag=f"yg{d}", name=f"yg{d}")
              for d in range(DC)]
        with tc.tile_pool(name="pF", bufs=2) as pF, \
             tc.tile_pool(name="pQ", bufs=2) as pQ, \
             tc.tile_pool(name="ps_y", bufs=1, space="PSUM") as ps_y, \
             tc.tile_pool(name="ps_f", bufs=2, space="PSUM") as ps_f:
            dt_ds = {}
            ub_ds = {}
            prep_u = {}

            def emit_prep_mm(d):
                # dt_raw matmuls + u = Exp(.+bias)
                u_t = pF.tile([128, L], F32, tag="u_t", bufs=1, name="u_t")
                for t2 in range(2):
                    pm = ps_f.tile([128, 1024], F32, tag="pa", name="pa")
                    for h in range(2):
                        o = 1024 * t2 + 512 * h
                        nc.tensor.matmul(
                            pm[:, 512 * h:512 * (h + 1)],
                            wdt_sb[:, 128 * d:128 * (d + 1)],
                            dtlow[:, o:o + 512], start=True, stop=True,
                            skip_group_check=True)
                    nc.scalar.activation(u_t[:, 1024 * t2:1024 * (t2 + 1)],
                                         pm[:], AF.Exp,
                                         bias=dtb_sb[:, d:d + 1])
                return u_t

            def emit_prep_ln(d, u_t):
                dt_d = pF.tile([128, L], F32R, tag="dt_d", bufs=2,
                               name="dt_d")
                nc.scalar.activation(dt_d[:], u_t[:], AF.Ln, bias=1.0)
                dt_ds[d] = dt_d

            def emit_prep_ub(d):
                ub_d = pF.tile([128, L], BF16, tag="ub_d", bufs=2, name="ub_d")
                nc.vector.tensor_mul(ub_d[:], dt_ds[d][:].bitcast(F32),
                                     xb[d][:])
                ub_sc = dram.tile([128, L], BF16, tag="ub_sc", bufs=2,
                                  name="ub_sc")
                nc.sync.dma_start(ub_sc[:], ub_d[:])
                ub_ds[d] = ub_sc

            def emit_prep(d):
                emit_prep_ln(d, emit_prep_mm(d))
                emit_prep_ub(d)

            emit_prep(0)

            NG = DC * SPC  # 64 global subtiles
            quads = {}
            a_ts = {}
            h_ts = {}
            hc_ts = {}
            ypsums = {}

            def emit_quad_reads(qg):
                # quad qg covers subtiles 4qg..4qg+3 of d = qg//4
                quad = pQ.tile([128, 4 * L], BF16, tag="quad", bufs=2)
                quad_v = quad[:].rearrange(
                    "(i n) (s t) -> i n s t", i=NSUB, s=4)
                src_r = ub_ds[qg // 4][32 * (qg % 4):32 * (qg % 4 + 1),
                                       :].rearrange("(s i) t -> i s t", s=4)
                for n in range(NST):
                    nc.sync.dma_start(quad_v[:, n, :, :], src_r)
                quads[qg] = quad

            def emit_S0(g):
                # a = exp(dt*A): PE replicate-matmul + ACT Exp
                d, sidx = divmod(g, SPC)
                dt_d = dt_ds[d]
                a_t = pF.tile([128, L], F32, tag="a", bufs=2)
                for t2 in range(2):
                    pm = ps_f.tile([128, 1024], F32, tag="pa")
                    for h in range(2):
                        o = 1024 * t2 + 512 * h
                        nc.tensor.matmul(
                            pm[:, 512 * h:512 * (h + 1)],
                            exp_sb[:, 128 * sidx:128 * (sidx + 1)],
                            dt_d[:, o:o + 512], start=True, stop=True,
                            skip_group_check=True)
                    nc.scalar.activation(
                        a_t[:, 1024 * t2:1024 * (t2 + 1)], pm[:], AF.Exp,
                        scale=asc_sb[:, d * SPC + sidx:d * SPC + sidx + 1])
                a_ts[g] = a_t

            def emit_S12(g):
                # b = ub_exp*B (DVE), h = scan (DVE), hc = h*C (Pool/DVE)
                sq = g % 4
                quad = quads[g // 4]
                b_t = pF.tile([128, L], BF16, tag="b")
                nc.vector.tensor_mul(b_t[:], quad[:, L * sq:L * (sq + 1)],
                                     bbc[:])
                h_t = pF.tile([128, L], BF16, tag="h")
                nc.vector.tensor_tensor_scan(h_t[:], a_ts.pop(g)[:], b_t[:],
                                             0.0, OP.mult, OP.add)
                hc = pF.tile([128, L], BF16, tag="hc")
                if g % 9 == 4:
                    nc.vector.tensor_mul(hc[:], h_t[:], cbc[:])
                else:
                    nc.gpsimd.tensor_mul(hc[:], h_t[:], cbc[:])
                hc_ts[g] = hc

            def emit_S4(g):
                d, sidx = divmod(g, SPC)
                if sidx == 0:
                    ypsums[d] = ps_y.tile([128, L], F32, tag="ypsum",
                                          name="ypsum")
                hc = hc_ts.pop(g)
                for tq in range(4):
                    nc.tensor.matmul(
                        ypsums[d][:, 512 * tq:512 * (tq + 1)],
                        sel_sb[:, 128 * sidx:128 * (sidx + 1)],
                        hc[:, 512 * tq:512 * (tq + 1)],
                        start=(sidx == 0), stop=(sidx == SPC - 1),
                        skip_group_check=True)
                if sidx == SPC - 1:
                    # y2 = D*x + y ;  yg = y2 * silu(z)
                    for hh in range(2):
                        o = 1024 * hh
                        y2 = pF.tile([128, 1024], F32, tag="y2", bufs=2)
                        nc.vector.scalar_tensor_tensor(
                            y2[:], xb[d][:, o:o + 1024], dpar_sb[:, d:d + 1],
                            ypsums[d][:, o:o + 1024], OP.mult, OP.add)
                        nc.vector.tensor_mul(yg[d][:, o:o + 1024], y2[:],
                                               zg[d][:, o:o + 1024])

            emit_quad_reads(0)
            for step in range(NG + 2):
                g0, g1, g2 = step, step - 1, step - 2
                if g0 < NG:
                    if g0 % SPC == 8 and g0 // SPC + 1 < DC:
                        emit_prep(g0 // SPC + 1)
                    gq = g0 + 2
                    if gq % 4 == 0 and gq < NG:
                        emit_quad_reads(gq // 4)
                    emit_S0(g0)
                if 0 <= g1 < NG:
                    emit_S12(g1)
                if 0 <= g2 < NG:
                    emit_S4(g2)
        # ====== Phase G: out_proj ======
        with tc.tile_pool(name="pG", bufs=3) as pG, \
             tc.tile_pool(name="ps_g", bufs=2, space="PSUM") as ps_g:
            for tb in range(RT):
                pm = ps_g.tile([128, D_MODEL], F32, tag="pmG")
                for d in range(DC):
                    for h in range(2):
                        nc.tensor.matmul(
                            pm[:, 512 * h:512 * (h + 1)],
                            yg[d][:, 128 * tb:128 * (tb + 1)],
                            wout_sb[d][:, 512 * h:512 * (h + 1)],
                            start=(d == 0), stop=(d == DC - 1))
                osb = pG.tile([128, D_MODEL], F32, tag="osb")
                nc.scalar.activation(osb[:], pm[:], AF.Copy)
                nc.sync.dma_start(out_part[128 * tb:128 * (tb + 1), :],
                                  osb[:])
        pY.release()
        pDE.release()
        pBC.release()
        cst.release()
        dram.release()
    nc.compile()

    return nc


_NC_CACHE = None


def _get_nc():
    global _NC_CACHE
    if _NC_CACHE is None:
        _NC_CACHE = _build()
    return _NC_CACHE


def kernel(input_ids=None, hidden_states=None, residual=None, norm_w=None,
           in_proj_w=None, conv_w=None, conv_b=None, x_proj_w=None,
           dt_proj_w=None, dt_proj_b=None, A_log=None, D_param=None,
           out_proj_w=None, **kwargs):
    import ml_dtypes
    bf16 = np.dtype(ml_dtypes.bfloat16)

    hs = np.asarray(hidden_states, np.float32)
    rs = np.asarray(residual, np.float32)
    ipw = np.asarray(in_proj_w, np.float32)
    cw = np.asarray(conv_w, np.float32)
    cb = np.asarray(conv_b, np.float32)
    xpw = np.asarray(x_proj_w, np.float32)
    dpw = np.asarray(dt_proj_w, np.float32)
    dpb = np.asarray(dt_proj_b, np.float32)
    al = np.asarray(A_log, np.float32)
    dpr = np.asarray(D_param, np.float32)
    opw = np.asarray(out_proj_w, np.float32)
    nw = np.asarray(norm_w, np.float32)

    r_full = hs + rs                               # host-side residual add

    def colpack(v):  # [DLOC] -> [128, DC], col d = v[d*128:(d+1)*128]
        return np.ascontiguousarray(v.reshape(DC, 128).T).astype(np.float32)

    selm = np.zeros((128, SPC * 128), np.float32)
    expm = np.zeros((128, SPC * 128), np.float32)
    for s in range(SPC):
        for i in range(NSUB):
            m = s * NSUB + i
            for n in range(NST):
                p = i * NST + n
                selm[p, s * 128 + m] = 1.0
                expm[m, s * 128 + p] = 1.0
    identb = np.eye(128, dtype=np.float32)

    nc = _get_nc()
    in_maps = []
    for c in range(N_CORES):
        b, k = c // TPG, c % TPG
        sl = slice(k * DLOC, (k + 1) * DLOC)
        slz = slice(D_INNER + k * DLOC, D_INNER + (k + 1) * DLOC)

        conv4 = cw[sl, 0, :]                       # [DLOC, 4]
        convd = np.zeros((128, DC * DCONV * 128), np.float32)
        for d in range(DC):
            for kk in range(DCONV):
                blk = DCONV * d + kk
                np.fill_diagonal(
                    convd[:, 128 * blk:128 * (blk + 1)],
                    conv4[128 * d:128 * (d + 1), kk])

        A = -np.exp(al[sl])                        # [DLOC, 16]
        a_sc = np.zeros((128, DC * SPC), np.float32)
        for d in range(DC):
            for s in range(SPC):
                rows = A[d * 128 + s * NSUB: d * 128 + (s + 1) * NSUB, :]
                a_sc[:, d * SPC + s] = rows.reshape(128)

        in_maps.append(dict(
            rin=r_full[b].astype(bf16),
            winx=np.ascontiguousarray(ipw[sl].T * nw[:, None]).astype(bf16),
            winz=np.ascontiguousarray(ipw[slz].T * nw[:, None]).astype(bf16),
            wxT=np.ascontiguousarray(xpw[:, sl].T).astype(bf16),
            wdtT=np.ascontiguousarray(dpw[sl].T).astype(bf16),
            woutT=np.ascontiguousarray(opw[:, sl].T).astype(bf16),
            convd=convd.astype(bf16),
            convb=colpack(cb[sl]),
            dtb=colpack(dpb[sl]),
            dpar=colpack(dpr[sl]),
            a_sc=a_sc,
            selm=selm.astype(bf16),
            expm=expm,
            identb=identb.astype(bf16),
        ))

    res = run_bass_kernel_spmd(nc, in_maps, core_ids=list(range(N_CORES)))
    outs = [res.results[c]["out_part"] for c in range(N_CORES)]
    full = np.stack([
        sum(outs[b * TPG + k] for k in range(TPG)) for b in range(BATCH)
    ]).astype(np.float32)
    return full
